# revision 21
# baseline (speedup 1.0000x reference)
# Trainium2 Bass kernel for nn_Attention_81028853007030 — v5 (Jacobi/scan,
# double-buffered).
#
# Model: 1-unit LSTM over [B=64, L=2048, E=300] -> scores -> (buggy) mask ->
# softmax over L -> attn * x.  Data-parallel over 8 cores (8 seqs each).
#
# Strategy:
#   - Layout: partition p = (s, k) = sequence s, 128-step chunk k. Each
#     partition holds one 128-timestep chunk: x_sb [128, 128, 300] f16.
#   - xg = x @ W_ih^T + b streamed per 8-timestep d-block as its DMA lands:
#     PE transpose (2 taus per PSUM bank) -> one [128,768] copy (alternating
#     DVE/ACT) -> gate matmuls -> bias STT into xg (g-major [128,4,128] f16).
#   - LSTM solved by Jacobi fixed-point iteration (NIT passes): gates use the
#     PREVIOUS iterate's h (shifted 1 step; chunk-boundary h/c via a PE
#     partition-shift matmul), then the c-recurrence is computed EXACTLY
#     within each chunk by one DVE tensor_tensor_scan. 5 passes -> ~5e-4
#     out error in fp16 (validated vs fp64 in proto_jacobi.py).
#   - Softmax stays in [V,T] layout; exp computed as sig(h)/(1-sig(h)) so the
#     ACT table never leaves the sigmoid set (no per-iteration table reload);
#     one block-diagonal [128x128] matmul sums the 16 chunks of each sequence
#     AND broadcasts the sum to all partitions. The (buggy) t=0 mask is a
#     host-precomputed per-partition additive mask.
#   - out = attn * x in place (per-tau scalar-ptr multiplies, 5 DVE : 3 ACT),
#     DMA out per 8-timestep block.
#   - x_sb/xg/attn are double-buffered and the For_i body runs TWO logical
#     iterations on alternating buffers, so iteration j+1's input DMA stream
#     fills the DMA engines while iteration j runs Jacobi/softmax/scale.
#     (GPSIMD tensor ops are avoided entirely: ~us-scale each on real HW.)

import os

import numpy as np

B, L, E = 64, 2048, 300
NCORES = 8
S = B // NCORES          # sequences per core
V = 128                  # partitions = S * 16 chunks of 128 timesteps
T = 128                  # timesteps per chunk
NIT = int(os.environ.get("KNIT", "5"))   # Jacobi passes (incl. h=0 pass)
NSDVE = int(os.environ.get("KSDVE", "5"))  # scale taus on DVE (of 8)
NCPDVE = int(os.environ.get("KCPDVE", "3"))  # xt copies on DVE (of 4)
ECH = [(0, 128), (128, 128), (256, 44)]  # E-chunks for the matmul
NBLK = 16                # 8-timestep blocks per chunk
UNROLL = 2               # logical iterations per For_i body

_CACHE = {}


def _build_nc(loop_n=0):
    from contextlib import ExitStack

    import concourse.bacc as bacc
    import concourse.mybir as mybir
    from concourse import tile
    from concourse.masks import make_identity

    F32 = mybir.dt.float32
    F16 = mybir.dt.float16
    Alu = mybir.AluOpType
    Act = mybir.ActivationFunctionType

    nc = bacc.Bacc("TRN2", target_bir_lowering=False, debug=False,
                   num_devices=NCORES)

    x_d = nc.dram_tensor("x", [S, L, E], F16, kind="ExternalInput")
    wt_d = nc.dram_tensor("wt", [3, 128, 4], F16, kind="ExternalInput")
    cst_d = nc.dram_tensor("cst", [128, 40], F32, kind="ExternalInput")
    pm_d = nc.dram_tensor("pm", [128, 256], F16, kind="ExternalInput")
    out_d = nc.dram_tensor("out", [S, L, E], F16, kind="ExternalOutput")

    x_v = x_d.ap().rearrange("s (k t) e -> (s k) t e", t=T)
    out_v = out_d.ap().rearrange("s (k t) e -> (s k) t e", t=T)

    with tile.TileContext(nc) as tc, ExitStack() as ctx:
        big = ctx.enter_context(tc.tile_pool(name="big", bufs=1))
        work = ctx.enter_context(tc.tile_pool(name="work", bufs=9))
        ppxt = ctx.enter_context(tc.tile_pool(name="ppxt", bufs=4, space="PSUM"))
        ppxg = ctx.enter_context(tc.tile_pool(name="ppxg", bufs=2, space="PSUM"))
        pps = ctx.enter_context(tc.tile_pool(name="pps", bufs=1, space="PSUM"))

        # ---- persistent tiles ----
        nb = UNROLL if loop_n else 1
        x_sbs = [big.tile([V, T, E], F16, tag=f"x_sb{u}", name=f"x_sb{u}")
                 for u in range(nb)]
        xgs = [big.tile([V, 4, T], F16, tag=f"xg{u}", name=f"xg{u}")
               for u in range(nb)]
        attns = [big.tile([V, T], F32, tag=f"attn{u}", name=f"attn{u}")
                 for u in range(nb)]
        ident = big.tile([128, 128], F16, tag="ident")
        wt_sb = big.tile([128, 3, 4], F16, tag="wt_sb")
        cst_sb = big.tile([128, 40], F32, tag="cst_sb")
        pm_sb = big.tile([128, 256], F16, tag="pm_sb")
        h4 = big.tile([V, 4, T], F16, tag="h4")
        p4 = big.tile([V, 4, T], F16, tag="p4")
        sig = big.tile([V, 3, T], F16, tag="sig")     # i, f, o
        tg = big.tile([V, T], F16, tag="tg")
        ig = big.tile([V, T], F16, tag="ig")
        cc = big.tile([V, T], F16, tag="cc")
        th = big.tile([V, T], F16, tag="th")
        hh = big.tile([V, T], F16, tag="hh")
        hc2 = big.tile([V, 2], F16, tag="hc2")
        tmp4 = big.tile([V, 4], F16, tag="tmp4")
        sgh = big.tile([V, T], F32, tag="sgh")
        omsg = big.tile([V, T], F32, tag="omsg")
        romsg = big.tile([V, T], F32, tag="romsg")
        expv = big.tile([V, T], F32, tag="expv")
        sume = big.tile([V, 1], F32, tag="sume")
        sume16 = big.tile([V, 1], F16, tag="sume16")
        rinv = big.tile([V, 1], F32, tag="rinv")

        bias32 = cst_sb[:, 0:32]     # b2 per (g, tau): repeat(b2, 8)
        w4row = cst_sb[:, 32:36]     # [128, 4], identical on every partition
        maskc = cst_sb[:, 36:37]     # additive t=0 mask (-60000 where sl>0)

        def emit_consts():
            make_identity(nc, ident[:])
            nc.sync.dma_start(wt_sb[:], wt_d.ap().rearrange("j p g -> p j g"))
            nc.sync.dma_start(cst_sb[:], cst_d.ap())
            nc.sync.dma_start(pm_sb[:], pm_d.ap())

        # ---- xg production ----
        def emit_T(x_sb, d, m):
            pst = ppxt.tile([128, 768], F16, tag="pst")
            for i in range(2):
                tau = d * 8 + 2 * m + i
                for j, (e0, cs) in enumerate(ECH):
                    nc.tensor.matmul(pst[0:cs, i * 384 + j * 128:
                                         i * 384 + (j + 1) * 128],
                                     lhsT=x_sb[:, tau, e0:e0 + cs],
                                     rhs=ident[:], is_transpose=True,
                                     start=True, stop=True)
            xt = work.tile([128, 768], F16, tag="xt")
            if m % 4 < NCPDVE:
                nc.vector.tensor_copy(out=xt[:], in_=pst[:])
            else:
                nc.scalar.copy(out=xt[:], in_=pst[:])
            return xt

        def emit_Gblock(xg, d, xts):
            pg = ppxg.tile([V, 32], F32, tag="pg")
            for m in range(4):
                for i in range(2):
                    q = 2 * m + i
                    for j, (e0, cs) in enumerate(ECH):
                        nc.tensor.matmul(pg[:, q * 4:(q + 1) * 4],
                                         lhsT=xts[m][0:cs,
                                                     i * 384 + j * 128:
                                                     i * 384 + (j + 1) * 128],
                                         rhs=wt_sb[0:cs, j, :],
                                         start=(j == 0), stop=(j == 2))
            nc.vector.scalar_tensor_tensor(
                xg[:, :, d * 8:(d + 1) * 8],
                in0=pg[:].rearrange("p (t g) -> p g t", g=4),
                scalar=1.0,
                in1=bias32.rearrange("p (g t) -> p g t", t=8),
                op0=Alu.mult, op1=Alu.add)

        def emit_body(u):
            x_sb, xg, attn = x_sbs[u], xgs[u], attns[u]
            # ---- input DMA + streamed xg ----
            for d in range(NBLK):
                nc.sync.dma_start(x_sb[:, d * 8:(d + 1) * 8, :],
                                  x_v[:, d * 8:(d + 1) * 8, :])
            prev = None
            for d in range(NBLK):
                cur = [emit_T(x_sb, d, m) for m in range(4)]
                if prev is not None:
                    emit_Gblock(xg, prev[0], prev[1])
                prev = (d, cur)
            emit_Gblock(xg, prev[0], prev[1])

            # ---- Jacobi passes ----
            for it in range(NIT):
                if it == 0:
                    gin = xg
                    cini = 0.0
                else:
                    # chunk-boundary h, c from previous pass via PE shift
                    nc.vector.tensor_copy(out=hc2[:, 0:1], in_=hh[:, T - 1:T])
                    nc.vector.tensor_copy(out=hc2[:, 1:2], in_=cc[:, T - 1:T])
                    sps = pps.tile([V, 2], F32, tag="sps")
                    nc.tensor.matmul(sps[:], lhsT=pm_sb[:, 0:128],
                                     rhs=hc2[:], start=True, stop=True)
                    # h4[g] = h * w4[g] ; p4 = xg + h4 shifted one step
                    for g in range(4):
                        nc.vector.tensor_scalar_mul(h4[:, g, :], hh[:],
                                                    w4row[:, g:g + 1])
                    nc.vector.tensor_tensor(out=p4[:, :, 1:T],
                                            in0=h4[:, :, 0:T - 1],
                                            in1=xg[:, :, 1:T], op=Alu.add)
                    nc.vector.tensor_tensor(
                        out=tmp4[:],
                        in0=sps[:, 0:1].broadcast_to([V, 4]),
                        in1=w4row[:], op=Alu.mult)
                    nc.vector.tensor_tensor(out=p4[:, :, 0], in0=tmp4[:],
                                            in1=xg[:, :, 0], op=Alu.add)
                    gin = p4
                    cini = sps[:, 1:2]
                nc.scalar.activation(sig[:], gin[:, 0:3, :], Act.Sigmoid)
                nc.scalar.activation(tg[:], gin[:, 3, :], Act.Tanh)
                nc.vector.tensor_tensor(out=ig[:], in0=sig[:, 0, :],
                                        in1=tg[:], op=Alu.mult)
                nc.vector.tensor_tensor_scan(out=cc[:], data0=sig[:, 1, :],
                                             data1=ig[:], initial=cini,
                                             op0=Alu.mult, op1=Alu.add)
                nc.scalar.activation(th[:], cc[:], Act.Tanh)
                nc.vector.tensor_tensor(out=hh[:], in0=th[:],
                                        in1=sig[:, 2, :], op=Alu.mult)

            # ---- softmax over L per sequence, in [V, T] layout ----
            # exp(h) = sig(h) / (1 - sig(h)) — stays in the sigmoid ACT set
            nc.vector.tensor_tensor(out=hh[:, 0:1], in0=hh[:, 0:1],
                                    in1=maskc, op=Alu.add)
            nc.scalar.activation(sgh[:], hh[:], Act.Sigmoid)
            nc.vector.tensor_scalar(omsg[:], sgh[:], -1.0, 1.0,
                                    Alu.mult, Alu.add)
            nc.vector.reciprocal(romsg[:], omsg[:])
            nc.vector.tensor_tensor(out=expv[:], in0=sgh[:], in1=romsg[:],
                                    op=Alu.mult)
            nc.vector.tensor_reduce(sume[:], expv[:],
                                    axis=mybir.AxisListType.X, op=Alu.add)
            nc.vector.tensor_copy(out=sume16[:], in_=sume[:])
            mps = pps.tile([V, 1], F32, tag="mps")
            nc.tensor.matmul(mps[:], lhsT=pm_sb[:, 128:256], rhs=sume16[:],
                             start=True, stop=True)
            nc.vector.reciprocal(rinv[:], mps[:])
            nc.vector.tensor_scalar_mul(attn[:], expv[:], rinv[:])

            # ---- out = attn * x (in place), then DMA out ----
            for d in range(NBLK):
                for tau in range(d * 8, (d + 1) * 8):
                    a = attn[:, tau:tau + 1]
                    xs = x_sb[:, tau, :]
                    if tau % 8 < NSDVE:
                        nc.vector.tensor_scalar_mul(xs, xs, a)
                    else:
                        nc.scalar.activation(xs, xs, Act.Copy, scale=a)
                nc.sync.dma_start(out_v[:, d * 8:(d + 1) * 8, :],
                                  x_sb[:, d * 8:(d + 1) * 8, :])

        emit_consts()
        if loop_n:
            with tc.For_i(0, loop_n, 1):
                for u in range(UNROLL):
                    emit_body(u)
        else:
            emit_body(0)

    nc.compile()
    return nc


def _get_nc(loop_n=0):
    key = ("nc", loop_n, NIT, NSDVE, NCPDVE)
    if key not in _CACHE:
        _CACHE[key] = _build_nc(loop_n)
    return _CACHE[key]


# gate order i,f,g,o -> i,f,o,g (sigmoid gates contiguous, tanh gate last)
_PERM = [0, 1, 3, 2]


def make_in_maps(x, source_lengths, W_ih, W_hh, b_ih, b_hh):
    x16 = np.asarray(x, dtype=np.float16)
    sl = np.asarray(source_lengths).astype(np.int64).reshape(B)
    wih = np.asarray(W_ih, dtype=np.float64)[_PERM]
    w4 = np.asarray(W_hh, dtype=np.float64).reshape(4)[_PERM]
    b2 = (np.asarray(b_ih, dtype=np.float64)
          + np.asarray(b_hh, dtype=np.float64))[_PERM]

    wt = np.zeros((3, 128, 4), dtype=np.float16)
    wt.reshape(384, 4)[0:E] = wih.T.astype(np.float16)

    # partition-shift matrix: out[m] = in[m-1], zero into chunk 0 of a seq
    pshift = np.zeros((128, 128), dtype=np.float16)
    for p in range(127):
        if (p + 1) % 16:
            pshift[p, p + 1] = 1.0
    # block-diagonal per-sequence sum+broadcast matrix
    msel = np.zeros((128, 128), dtype=np.float16)
    for p in range(128):
        msel[p, (p // 16) * 16:(p // 16) * 16 + 16] = 1.0
    pm = np.concatenate([pshift, msel], axis=1)

    in_maps = []
    for cidx in range(NCORES):
        cst = np.zeros((128, 40), dtype=np.float32)
        cst[:, 0:32] = np.repeat(b2.astype(np.float32), 8)
        cst[:, 32:36] = w4.astype(np.float32)
        slc = sl[cidx * S:(cidx + 1) * S]
        mask = np.zeros((128,), dtype=np.float32)
        for s in range(S):
            if slc[s] > 0:
                mask[s * 16] = -60000.0
        cst[:, 36] = mask
        in_maps.append({
            "x": np.ascontiguousarray(x16[cidx * S:(cidx + 1) * S]),
            "wt": wt,
            "cst": cst,
            "pm": pm,
        })
    return in_maps


def kernel(x, source_lengths, W_ih, W_hh, b_ih, b_hh):
    from concourse.bass_utils import run_bass_kernel_spmd

    nc = _get_nc()
    in_maps = make_in_maps(x, source_lengths, W_ih, W_hh, b_ih, b_hh)
    res = run_bass_kernel_spmd(nc, in_maps, core_ids=list(range(NCORES)))
    out = np.concatenate(
        [res.results[c]["out"].astype(np.float32) for c in range(NCORES)],
        axis=0)
    return out
